# revision 6
# baseline (speedup 1.0000x reference)
"""DiagSSMBlock Trainium2 kernel.

h_t = sum_{k=0..t} a^k * (B^T x_{t-k})  ==  h_t = a * h_{t-1} + s_t, s = B^T x^T.

Strategy: shard T across the 8 cores (1024 steps each + 32-step halo; |a| <=
sqrt(2/1024) ~ 0.044 so a^32 < 1e-43 == 0 in fp32, making slabs exactly
independent).  Host passes x pre-transposed ([H, T_slab]) so the tensor engine
can contract over H with no on-chip transposes; the scan output is returned
channel-major [H, T_slab] and transposed back on host.

Per core: DMA B + xT slab -> 8x K-chunked fp32r matmul accumulation into PSUM
(3 chunks of 352 time-cols) -> tensor_tensor_scan (the SSM recurrence) per
128-channel group -> DMA out.

Perf structure: dummy warm-up matmuls lift the PE HAM clock-gate to 2.4 GHz
during the input-DMA ramp; DMA issue is spread across both HWDGE engines
(sync + scalar); the matmul loop runs chunk-outer / group-inner so the input
byte-demand curve stays under HBM bandwidth (group-outer needs all of xT in
the first ~10us, which does not fit).
"""

import sys

if "/opt/trn_rl_repo" not in sys.path:
    sys.path.insert(0, "/opt/trn_rl_repo")

import numpy as np

T, H = 8192, 1024
NC = 8
P = 128
T_LOC = T // NC            # 1024 output timesteps per core
HALO = 32                  # scan warmup; a^32 == 0 in fp32
W = T_LOC + HALO           # 1056
CH = 352                   # psum chunk width (3 chunks of 352 = 1056)
NCHUNK = W // CH
KQ = H // P                # 8 contraction chunks
G = H // P                 # 8 channel groups
N_WARM = 8                 # dummy matmuls to lift the HAM clock gate

MM_DTYPE = "float32r"      # matmul operand dtype: "float32" (4 cyc/row) or
                           # "float32r" (1 cyc/row at N>=256)

_state = {}


def _build_nc():
    import concourse.tile as tile
    from concourse import bacc, mybir

    mm_dt = getattr(mybir.dt, MM_DTYPE)
    f32 = mybir.dt.float32

    nc = bacc.Bacc("TRN2", target_bir_lowering=False, debug=False, num_devices=NC)
    xt_e = nc.dram_tensor("xt", [H, W], mm_dt, kind="ExternalInput").ap()
    b_e = nc.dram_tensor("b", [H, H], mm_dt, kind="ExternalInput").ap()
    av_e = nc.dram_tensor("av", [P, G], f32, kind="ExternalInput").ap()
    out_e = nc.dram_tensor("out", [H, T_LOC], f32, kind="ExternalOutput").ap()
    flush_e = nc.dram_tensor("warm_flush", [P, 1], f32).ap()

    with tile.TileContext(nc) as tc:
        with (
            tc.tile_pool(name="consts", bufs=1) as consts,
            tc.tile_pool(name="bpool", bufs=1) as bpool,
            tc.tile_pool(name="xpool", bufs=1) as xpool,
            tc.tile_pool(name="hpool", bufs=1) as hpool,
            tc.tile_pool(name="pspool", bufs=6, space="PSUM") as pspool,
            tc.tile_pool(name="warmps", bufs=1, space="PSUM") as warmps,
        ):
            # PE warm-up: dummy fp32 matmuls on a zeroed scratch tile, gated
            # only on a gpsimd memset, so the HAM clock-gate lifts to 2.4 GHz
            # during the input-DMA ramp.  The flush chain (copy + tiny DMA to
            # an internal DRAM tensor) keeps it live through DCE and stays
            # entirely on gpsimd so no other engine queue blocks on it.
            warm_sb = consts.tile([P, P], f32, tag="warm")
            nc.gpsimd.memset(warm_sb[:], 0.0)
            wps = warmps.tile([P, P], f32)
            for i in range(N_WARM):
                nc.tensor.matmul(
                    wps[:],
                    warm_sb[:],
                    warm_sb[:],
                    start=(i == 0),
                    stop=(i == N_WARM - 1),
                )

            # a broadcast tiles: memset 1.0 on gpsimd, scaled per-partition on
            # DVE (fast there, dog-slow on gpsimd).
            av_sb = consts.tile([P, G], f32, tag="av")
            nc.sync.dma_start(av_sb[:], av_e[:])
            a_bc = []
            for g in range(G):
                t = consts.tile([P, CH], f32, tag=f"abc{g}")
                nc.gpsimd.memset(t[:], 1.0)
                nc.vector.tensor_scalar_mul(t[:], t[:], av_sb[:, g : g + 1])
                a_bc.append(t)

            # Input loads.  sync: xt chunks 0/2; scalar: b (16 half-tiles),
            # xt chunk 1, then output stores.
            xt_sb = [[None] * NCHUNK for _ in range(KQ)]
            for ni in (0, 2):
                n0 = ni * CH
                for kq in range(KQ):
                    xtile = xpool.tile([P, CH], mm_dt, tag=f"x{kq}_{ni}")
                    nc.sync.dma_start(
                        xtile[:], xt_e[kq * P : (kq + 1) * P, n0 : n0 + CH]
                    )
                    xt_sb[kq][ni] = xtile
            b_sb = [[None, None] for _ in range(KQ)]
            for half in range(2):
                for kq in range(KQ):
                    bt = bpool.tile([P, H // 2], mm_dt, tag=f"b{kq}_{half}")
                    nc.scalar.dma_start(
                        bt[:],
                        b_e[kq * P : (kq + 1) * P, half * (H // 2) : (half + 1) * (H // 2)],
                    )
                    b_sb[kq][half] = bt
            for kq in range(KQ):
                ni = 1
                xtile = xpool.tile([P, CH], mm_dt, tag=f"x{kq}_{ni}")
                nc.scalar.dma_start(
                    xtile[:], xt_e[kq * P : (kq + 1) * P, CH : 2 * CH]
                )
                xt_sb[kq][ni] = xtile

            def b_slice(kq, g):
                half, off = divmod(g * P, H // 2)
                return b_sb[kq][half][:, off : off + P]

            # Matmul + scan: chunk-outer / group-inner keeps the input-byte
            # demand curve under HBM bandwidth.
            h_t = []
            for g in range(G):
                hg = hpool.tile([P, W], f32, tag=f"h{g}")
                h_t.append(hg)
            for ni in range(NCHUNK):
                n0 = ni * CH
                for g in range(G):
                    ps = pspool.tile([P, CH], f32)
                    for kq in range(KQ):
                        nc.tensor.matmul(
                            ps[:],
                            b_slice(kq, g),
                            xt_sb[kq][ni][:],
                            start=(kq == 0),
                            stop=(kq == KQ - 1),
                        )
                    init = 0.0 if ni == 0 else h_t[g][:, n0 - 1 : n0]
                    nc.vector.tensor_tensor_scan(
                        h_t[g][:, n0 : n0 + CH],
                        a_bc[g][:],
                        ps[:],
                        init,
                        op0=mybir.AluOpType.mult,
                        op1=mybir.AluOpType.add,
                    )
                    if ni == NCHUNK - 1:
                        nc.scalar.dma_start(
                            out_e[g * P : (g + 1) * P, :], h_t[g][:, HALO:W]
                        )

            # warm-up flush: emitted last so the PSUM read (not legal on
            # gpsimd) sits at the tail of the DVE queue and blocks nothing.
            flush_sb = consts.tile([P, 1], f32, tag="flush")
            nc.vector.tensor_copy(flush_sb[:], wps[:, 0:1])
            nc.gpsimd.dma_start(flush_e[:], flush_sb[:])

    nc.compile()
    return nc


def _get_nc():
    if "nc" not in _state:
        _state["nc"] = _build_nc()
    return _state["nc"]


def _shard_inputs(x_seq, a_diag, b_mat):
    x = np.asarray(x_seq, dtype=np.float32)
    a = np.asarray(a_diag, dtype=np.float32)
    b = np.ascontiguousarray(np.asarray(b_mat, dtype=np.float32))
    x_pad = np.concatenate([np.zeros((HALO, H), np.float32), x], axis=0)
    xT = np.ascontiguousarray(x_pad.T)  # [H, T + HALO]
    av = np.ascontiguousarray(a.reshape(G, P).T)  # [P, G]
    in_maps = []
    for i in range(NC):
        in_maps.append(
            {
                "xt": np.ascontiguousarray(xT[:, i * T_LOC : i * T_LOC + W]),
                "b": b,
                "av": av,
            }
        )
    return in_maps


def kernel(x_seq, a_diag, b_mat):
    from concourse.bass_utils import run_bass_kernel_spmd

    nc = _get_nc()
    in_maps = _shard_inputs(x_seq, a_diag, b_mat)
    res = run_bass_kernel_spmd(nc, in_maps, list(range(NC)))
    _state["last_result"] = res
    out = np.concatenate(
        [np.asarray(res.results[i]["out"]).T for i in range(NC)], axis=0
    )
    return out


# revision 7
# speedup vs baseline: 1.0281x; 1.0281x over previous
"""DiagSSMBlock Trainium2 kernel.

h_t = sum_{k=0..t} a^k * (B^T x_{t-k})  ==  h_t = a * h_{t-1} + s_t, s = B^T x^T.

Strategy: shard T across the 8 cores (1024 steps each + 32-step halo; |a| <=
sqrt(2/1024) ~ 0.044 so a^32 < 1e-43 == 0 in fp32, making slabs exactly
independent).  Host passes x pre-transposed ([H, T_slab]) so the tensor engine
can contract over H with no on-chip transposes; the scan output is returned
channel-major [H, T_slab] and transposed back on host.

Per core: DMA B + xT slab -> 8x K-chunked fp32r matmul accumulation into PSUM
(3 chunks of 352 time-cols) -> tensor_tensor_scan (the SSM recurrence) per
128-channel group -> DMA out.

Perf structure: dummy warm-up matmuls lift the PE HAM clock-gate to 2.4 GHz
during the input-DMA ramp; DMA issue is spread across both HWDGE engines
(sync + scalar); the matmul loop runs chunk-outer / group-inner so the input
byte-demand curve stays under HBM bandwidth (group-outer needs all of xT in
the first ~10us, which does not fit).
"""

import sys

if "/opt/trn_rl_repo" not in sys.path:
    sys.path.insert(0, "/opt/trn_rl_repo")

import numpy as np

T, H = 8192, 1024
NC = 8
P = 128
T_LOC = T // NC            # 1024 output timesteps per core
HALO = 32                  # scan warmup; a^32 == 0 in fp32
W = T_LOC + HALO           # 1056
CH = 352                   # psum chunk width (3 chunks of 352 = 1056)
NCHUNK = W // CH
KQ = H // P                # 8 contraction chunks
G = H // P                 # 8 channel groups
N_WARM = 8                 # dummy matmuls to lift the HAM clock gate

MM_DTYPE = "float32r"      # matmul operand dtype: "float32" (4 cyc/row) or
                           # "float32r" (1 cyc/row at N>=256)

_state = {}


def _build_nc():
    import concourse.bass as bass_mod
    import concourse.tile as tile
    from concourse import bacc, mybir

    mm_dt = getattr(mybir.dt, MM_DTYPE)
    f32 = mybir.dt.float32

    nc = bacc.Bacc("TRN2", target_bir_lowering=False, debug=False, num_devices=NC)
    xt_e = nc.dram_tensor("xt", [H, W], mm_dt, kind="ExternalInput").ap()
    b_e = nc.dram_tensor("b", [H, H], mm_dt, kind="ExternalInput").ap()
    av_e = nc.dram_tensor("av", [P, G], f32, kind="ExternalInput").ap()
    out_e = nc.dram_tensor("out", [H, T_LOC], f32, kind="ExternalOutput").ap()
    flush_e = nc.dram_tensor("warm_flush", [P, 1], f32).ap()

    with tile.TileContext(nc) as tc:
        with (
            tc.tile_pool(name="consts", bufs=1) as consts,
            tc.tile_pool(name="bpool", bufs=1) as bpool,
            tc.tile_pool(name="xpool", bufs=1) as xpool,
            tc.tile_pool(name="hpool", bufs=1) as hpool,
            tc.tile_pool(name="pspool", bufs=6, space="PSUM") as pspool,
            tc.tile_pool(name="warmps", bufs=1, space="PSUM") as warmps,
        ):
            # PE warm-up: dummy fp32 matmuls on a zeroed scratch tile, gated
            # only on a gpsimd memset, so the HAM clock-gate lifts to 2.4 GHz
            # during the input-DMA ramp.  The flush chain (copy + tiny DMA to
            # an internal DRAM tensor) keeps it live through DCE and stays
            # entirely on gpsimd so no other engine queue blocks on it.
            warm_sb = consts.tile([P, P], f32, tag="warm")
            nc.gpsimd.memset(warm_sb[:], 0.0)
            wps = warmps.tile([P, P], f32)
            last_mm = None
            for i in range(N_WARM):
                last_mm = nc.tensor.matmul(
                    wps[:],
                    warm_sb[:],
                    warm_sb[:],
                    start=(i == 0),
                    stop=(i == N_WARM - 1),
                )

            # a broadcast tiles: memset 1.0 on gpsimd, scaled per-partition on
            # DVE (fast there, dog-slow on gpsimd).
            av_sb = consts.tile([P, G], f32, tag="av")
            nc.sync.dma_start(av_sb[:], av_e[:])
            a_bc = []
            for g in range(G):
                t = consts.tile([P, CH], f32, tag=f"abc{g}")
                nc.gpsimd.memset(t[:], 1.0)
                nc.vector.tensor_scalar_mul(t[:], t[:], av_sb[:, g : g + 1])
                a_bc.append(t)

            # Input loads.  sync: xt chunks 0/2; scalar: b (16 half-tiles),
            # xt chunk 1, then output stores.
            xt_sb = [[None] * NCHUNK for _ in range(KQ)]
            for ni in (0, 2):
                n0 = ni * CH
                for kq in range(KQ):
                    xtile = xpool.tile([P, CH], mm_dt, tag=f"x{kq}_{ni}")
                    nc.sync.dma_start(
                        xtile[:], xt_e[kq * P : (kq + 1) * P, n0 : n0 + CH]
                    )
                    xt_sb[kq][ni] = xtile
            b_sb = [[None, None] for _ in range(KQ)]
            for half in range(2):
                for kq in range(KQ):
                    bt = bpool.tile([P, H // 2], mm_dt, tag=f"b{kq}_{half}")
                    nc.scalar.dma_start(
                        bt[:],
                        b_e[kq * P : (kq + 1) * P, half * (H // 2) : (half + 1) * (H // 2)],
                    )
                    b_sb[kq][half] = bt
            for kq in range(KQ):
                ni = 1
                xtile = xpool.tile([P, CH], mm_dt, tag=f"x{kq}_{ni}")
                nc.scalar.dma_start(
                    xtile[:], xt_e[kq * P : (kq + 1) * P, CH : 2 * CH]
                )
                xt_sb[kq][ni] = xtile

            def b_slice(kq, g):
                half, off = divmod(g * P, H // 2)
                return b_sb[kq][half][:, off : off + P]

            # Matmul + scan: chunk-outer / group-inner keeps the input-byte
            # demand curve under HBM bandwidth.
            h_t = []
            for g in range(G):
                hg = hpool.tile([P, W], f32, tag=f"h{g}")
                h_t.append(hg)
            for ni in range(NCHUNK):
                n0 = ni * CH
                for g in range(G):
                    ps = pspool.tile([P, CH], f32)
                    for kq in range(KQ):
                        mm = nc.tensor.matmul(
                            ps[:],
                            b_slice(kq, g),
                            xt_sb[kq][ni][:],
                            start=(kq == 0),
                            stop=(kq == KQ - 1),
                        )
                        if kq == 0 and last_mm is not None:
                            # pin PE group execution order = emission order so
                            # scans fire promptly and psum slots recycle
                            bass_mod._add_dep_helper(
                                mm.ins, last_mm.ins, False, "PE group order"
                            )
                        last_mm = mm
                    init = 0.0 if ni == 0 else h_t[g][:, n0 - 1 : n0]
                    nc.vector.tensor_tensor_scan(
                        h_t[g][:, n0 : n0 + CH],
                        a_bc[g][:],
                        ps[:],
                        init,
                        op0=mybir.AluOpType.mult,
                        op1=mybir.AluOpType.add,
                    )
                    if ni == NCHUNK - 1:
                        nc.sync.dma_start(
                            out_e[g * P : (g + 1) * P, :], h_t[g][:, HALO:W]
                        )

            # warm-up flush: emitted last so the PSUM read (not legal on
            # gpsimd) sits at the tail of the DVE queue and blocks nothing.
            flush_sb = consts.tile([P, 1], f32, tag="flush")
            nc.vector.tensor_copy(flush_sb[:], wps[:, 0:1])
            nc.gpsimd.dma_start(flush_e[:], flush_sb[:])

    nc.compile()
    return nc


def _get_nc():
    if "nc" not in _state:
        _state["nc"] = _build_nc()
    return _state["nc"]


def _shard_inputs(x_seq, a_diag, b_mat):
    x = np.asarray(x_seq, dtype=np.float32)
    a = np.asarray(a_diag, dtype=np.float32)
    b = np.ascontiguousarray(np.asarray(b_mat, dtype=np.float32))
    x_pad = np.concatenate([np.zeros((HALO, H), np.float32), x], axis=0)
    xT = np.ascontiguousarray(x_pad.T)  # [H, T + HALO]
    av = np.ascontiguousarray(a.reshape(G, P).T)  # [P, G]
    in_maps = []
    for i in range(NC):
        in_maps.append(
            {
                "xt": np.ascontiguousarray(xT[:, i * T_LOC : i * T_LOC + W]),
                "b": b,
                "av": av,
            }
        )
    return in_maps


def kernel(x_seq, a_diag, b_mat):
    from concourse.bass_utils import run_bass_kernel_spmd

    nc = _get_nc()
    in_maps = _shard_inputs(x_seq, a_diag, b_mat)
    res = run_bass_kernel_spmd(nc, in_maps, list(range(NC)))
    _state["last_result"] = res
    out = np.concatenate(
        [np.asarray(res.results[i]["out"]).T for i in range(NC)], axis=0
    )
    return out
